# revision 1
# baseline (speedup 1.0000x reference)
"""Causal multi-head self-attention on 8 Trainium2 NeuronCores.

Problem: B=4, S=2048, D_MODEL=2048, H=16 heads, d_k=128, RoPE, causal
softmax, fp32 I/O.

Sharding: 8 cores = 4 batches x 2 head-groups (8 heads each).  Each core
computes QKV projections for its head group (weights sharded by output
rows), RoPE, head-local causal attention, and a partial o_proj over its
1024 input features.  The host sums the two partial o_proj outputs per
batch (the unshard step for the K-sharded o_proj matmul).

Kernel layout notes (per core):
- All matmul operands are float32r (full PE rate at moving dim >= 256).
- x, weights are pre-transposed host-side so every matmul's contraction
  dim lands on the partition axis with no on-device transposes at all.
- Q,K are produced head-transposed (QT/KT: [d_k, S]); scores are computed
  transposed (scoresT [k, q]) so softmax denominators come from a
  ones-vector matmul and probs feed the PV matmul directly (softmax skips
  the max subtraction: causal logits here are ~N(0,1), exp is safe).
- RoPE pairs are DE-INTERLEAVED via a host-side permutation of the wq/wk
  output columns (QK^T is invariant to a shared row permutation), making
  RoPE six contiguous half-tile DVE ops off PSUM -- no row swaps.
- V is projected x-stationary straight into [s, d] layout (no transposes).
- Causal: future 128-k chunks are skipped; diagonal chunks compute only
  the valid q-suffix, with one [128,128] triangle mask tile.
- Scratch (QT/KT/V) bounces through DRAM; writes ride the SWDGE (gpsimd)
  queue so they never head-of-line-block input loads on the HWDGE queue.
"""

import sys

for _p in ("/opt/trn_rl_repo", "/root/.axon_site/_ro/trn_rl_repo"):
    if _p not in sys.path:
        sys.path.insert(0, _p)

import numpy as np

import concourse.bacc as bacc
import concourse.mybir as mybir
import concourse.tile as tile

F32 = mybir.dt.float32
F32R = mybir.dt.float32r
EXPF = mybir.ActivationFunctionType.Exp
COPYF = mybir.ActivationFunctionType.Copy
MUL = mybir.AluOpType.mult
ADD = mybir.AluOpType.add
SUB = mybir.AluOpType.subtract

D_MODEL = 2048
NUM_HEADS = 16
D_K = 128
ROPE_THETA = 10000.0
B = 4
S = 2048
N_CORES = 8
GROUPS = 2  # head groups (tensor parallel factor)
H_LOC = NUM_HEADS // GROUPS  # heads per core


def build_nc(D, S_, H_loc, QB=512):
    """Build the per-core Bass program. Parametric for small-size sim tests."""
    P = 128
    DK = 128
    E = H_loc * DK  # local qkv output features
    KCN = D // P  # contraction chunks for projections
    NSB = S_ // P  # 128-token blocks
    NQB = S_ // QB  # q blocks in attention
    NDIAG = QB // P  # diagonal 128-k chunks per q block
    SCALE = 1.0 / float(np.sqrt(DK))

    nc = bacc.Bacc("TRN2", target_bir_lowering=False, debug=False,
                   num_devices=N_CORES)

    xT = nc.dram_tensor("xT", [D, S_], F32, kind="ExternalInput")
    wqT = nc.dram_tensor("wqT", [D, E], F32, kind="ExternalInput")
    wkT = nc.dram_tensor("wkT", [D, E], F32, kind="ExternalInput")
    wvT = nc.dram_tensor("wvT", [D, E], F32, kind="ExternalInput")
    woT = nc.dram_tensor("woT", [E, D], F32, kind="ExternalInput")
    # RoPE tables for the DE-INTERLEAVED head layout (even dims in rows
    # 0..63, odd dims in rows 64..127 -- wq/wk columns are permuted
    # host-side, which leaves Q.K dot products invariant).
    # duplicated to full d_k height so both halves have base-0 AND base-64
    # slices (SB-SB tensor_tensor inputs must share a base partition)
    cosH = nc.dram_tensor("cosH", [DK, S_], F32, kind="ExternalInput")
    sinH = nc.dram_tensor("sinH", [DK, S_], F32, kind="ExternalInput")
    masks = nc.dram_tensor("masks", [P, P], F32, kind="ExternalInput")
    ones_in = nc.dram_tensor("ones", [P, 1], F32, kind="ExternalInput")
    out = nc.dram_tensor("out", [S_, D], F32, kind="ExternalOutput")

    xT_t = xT.rearrange("(kc p) s -> p kc s", p=P).bitcast(F32R)
    wT_t = {
        "q": wqT.rearrange("(kc p) e -> p kc e", p=P).bitcast(F32R),
        "k": wkT.rearrange("(kc p) e -> p kc e", p=P).bitcast(F32R),
        "v": wvT.rearrange("(kc p) e -> p kc e", p=P).bitcast(F32R),
    }

    with tile.TileContext(nc) as tc:
        with (
            tc.tile_pool(name="dram", bufs=1, space="DRAM") as dram,
            tc.tile_pool(name="const", bufs=1) as const,
        ):
            # DRAM scratch for rotated QT/KT ([h, dk, S]) and V ([sb, s, e])
            qt_dram = dram.tile([H_loc, DK, S_], F32R)
            kt_dram = dram.tile([H_loc, DK, S_], F32R)
            v_dram = dram.tile([NSB, P, H_loc * DK], F32R)

            ones_sb = const.tile([P, 1], F32R)
            nc.sync.dma_start(ones_sb[:], ones_in[:].bitcast(F32R))

            # ---------------- Phase 1: projections + RoPE -----------------
            NST = S_ // 512  # 512-wide s tiles
            with (
                tc.tile_pool(name="xres", bufs=1) as xres,
                tc.tile_pool(name="trig", bufs=1) as trig,
            ):
                x_res = xres.tile([P, KCN, S_], F32R)

                cos_sb = trig.tile([DK, S_], F32)
                sin_sb = trig.tile([DK, S_], F32)

                # --- Phase 1b: V via x-stationary matmuls (direct [s, d]) ---
                # Features are processed in 512-wide halves so the half of wvT
                # in use stays SBUF-resident (full wvT + resident x would
                # exceed SBUF).  8 s-blocks accumulate per PSUM round.
                with (
                    tc.tile_pool(name="wv", bufs=1) as wvp,
                    tc.tile_pool(name="v_ps", bufs=8, space="PSUM") as v_ps,
                    tc.tile_pool(name="vout", bufs=4) as voutp,
                ):
                    EH = min(512, E)
                    # all groups 4 banks: consecutive groups double-buffer
                    # in PSUM (4+4), so group-boundary evictions overlap the
                    # next group's matmuls
                    gsz0 = min(4, NSB)
                    group_sizes = [gsz0] * (NSB // gsz0)
                    for half in range(E // EH):
                        e_lo = half * EH
                        wv_res = wvp.tile([P, KCN, EH], F32R, tag="wv",
                                          name=f"wv_{half}")
                        if half != 0:
                            for kc in range(KCN):
                                nc.sync.dma_start(
                                    wv_res[:, kc],
                                    wT_t["v"][:, kc, e_lo:e_lo + EH],
                                )
                        sb0 = 0
                        for gi, gsz in enumerate(group_sizes):
                            psv = []
                            for i in range(gsz):
                                psv.append(v_ps.tile(
                                    [P, EH], F32, tag="vps",
                                    name=f"vps_{half}_{gi}_{i}"))
                            for kc in range(KCN):
                                if half == 0 and gi == 0:
                                    # V runs first: stream x and its weights
                                    # together so matmuls start immediately
                                    nc.sync.dma_start(
                                        wv_res[:, kc],
                                        wT_t["v"][:, kc, e_lo:e_lo + EH],
                                    )
                                    nc.sync.dma_start(x_res[:, kc], xT_t[:, kc])
                                for i in range(gsz):
                                    sb_i = sb0 + i
                                    nc.tensor.matmul(
                                        psv[i][:],
                                        x_res[:, kc, sb_i * P:(sb_i + 1) * P],
                                        wv_res[:, kc],
                                        start=(kc == 0),
                                        stop=(kc == KCN - 1),
                                    )
                            for i in range(gsz):
                                sb_i = sb0 + i
                                v_sb = voutp.tile([P, EH], F32R, tag="vout")
                                nc.scalar.activation(v_sb[:], psv[i][:], COPYF)
                                nc.gpsimd.dma_start(
                                    v_dram[sb_i, :, e_lo:e_lo + EH], v_sb[:]
                                )
                            sb0 += gsz

                # --- Phase 1a: Q/K head-transposed projections + RoPE ---
                with (
                    tc.tile_pool(name="wslice", bufs=6) as wslice,
                    tc.tile_pool(name="qk_ps", bufs=2, space="PSUM") as qk_ps,
                    tc.tile_pool(name="ropet", bufs=1) as ropet,
                    tc.tile_pool(name="rawp", bufs=1) as rawp,
                    tc.tile_pool(name="rotE_p", bufs=2) as rotEp,
                    tc.tile_pool(name="rotO_p", bufs=1) as rotOp,
                ):
                    HH = DK // 2
                    nc.sync.dma_start(cos_sb[:], cosH[:])
                    nc.sync.dma_start(sin_sb[:], sinH[:])
                    for t in ("q", "k"):
                        for h in range(H_loc):
                            # 4 psum banks as one group; released by RoPE reads
                            pgrp = qk_ps.tile([P, NST, 512], F32, tag="qk",
                                              name=f"pg_{t}_{h}")
                            for kc in range(KCN):
                                w_sl = wslice.tile([P, DK], F32R, tag="wsl")
                                nc.sync.dma_start(
                                    w_sl[:], wT_t[t][:, kc, h * DK:(h + 1) * DK]
                                )
                                for st in range(NST):
                                    nc.tensor.matmul(
                                        pgrp[:, st],
                                        w_sl[:],
                                        x_res[:, kc, st * 512:(st + 1) * 512],
                                        start=(kc == 0),
                                        stop=(kc == KCN - 1),
                                    )
                            # RoPE, de-interleaved: rows 0..63 = even dims E,
                            # rows 64..127 = odd dims O (w cols permuted on
                            # host).  rot_E = E*cos - O*sin; rot_O = E*sin +
                            # O*cos.  ACT (idle here) evicts PSUM once; the six
                            # DVE multiplies then run SBUF-only at ~2x the
                            # PSUM-read rate, and the PSUM group releases
                            # after just the eviction.  Each SB-SB input pair
                            # shares a base partition (tables are full-height
                            # duplicates); out bases are unconstrained.
                            raw = rawp.tile([DK, S_], F32, tag="raw")
                            raw_v = raw[:].rearrange("p (a b) -> p a b", b=512)
                            nc.scalar.activation(raw_v, pgrp[:], COPYF)
                            rotE = rotEp.tile([HH, S_], F32R, tag="rotE")
                            rotO = rotOp.tile([HH, S_], F32R, tag="rotO")
                            tmp = ropet.tile([HH, S_], F32, tag="tmp")
                            nc.vector.tensor_tensor(
                                rotE[:], raw[:HH], cos_sb[:HH], MUL)
                            nc.vector.tensor_tensor(
                                tmp[:], raw[HH:], sin_sb[HH:], MUL)
                            nc.vector.tensor_tensor(
                                rotE[:], rotE[:], tmp[:], SUB)
                            nc.vector.tensor_tensor(
                                rotO[:], raw[:HH], sin_sb[:HH], MUL)
                            tmp_b = ropet.tile([HH, S_], F32, tag="tmp")
                            nc.vector.tensor_tensor(
                                tmp_b[:], raw[HH:], cos_sb[HH:], MUL)
                            nc.vector.tensor_tensor(
                                rotO[:], rotO[:], tmp_b[:], ADD)
                            dst = qt_dram if t == "q" else kt_dram
                            # final head drains via the idle fast SP queue so
                            # the attention pools' SBUF space frees sooner
                            eng = nc.sync if (t == "k" and h == H_loc - 1) \
                                else nc.gpsimd
                            eng.dma_start(dst[h, :HH], rotE[:])
                            eng.dma_start(dst[h, HH:], rotO[:])

            # ---------------- Phase 2: attention -----------------
            with (
                tc.tile_pool(name="attnT", bufs=1) as attnT_pool,
                tc.tile_pool(name="wo", bufs=1) as wo_pool,
            ):
                attnT = attnT_pool.tile([DK, H_loc, S_], F32R)
                wo_sb = wo_pool.tile([P, H_loc, D], F32R)
                woT_t = woT.rearrange("(ec p) n -> p ec n", p=P).bitcast(F32R)
                with (
                    tc.tile_pool(name="heads", bufs=2) as heads,
                    tc.tile_pool(name="mask", bufs=1) as maskp,
                    tc.tile_pool(name="expt", bufs=4) as expt,
                    tc.tile_pool(name="sc_ps", bufs=4, space="PSUM") as sc_ps,
                    tc.tile_pool(name="den_ps", bufs=2, space="PSUM") as den_ps,
                    tc.tile_pool(name="pv_ps", bufs=2, space="PSUM") as pv_ps,
                    tc.tile_pool(name="inv", bufs=2) as invp,
                ):

                    mask_sb = maskp.tile([P, P], F32R)
                    nc.sync.dma_start(mask_sb[:], masks[:].bitcast(F32R))

                    for h in range(H_loc):
                        qt_sb = heads.tile([DK, S_], F32R, tag="qt")
                        kt_sb = heads.tile([DK, S_], F32R, tag="kt")
                        v_sb = heads.tile([P, NSB, DK], F32R, tag="v")
                        nc.sync.dma_start(qt_sb[:], qt_dram[h])
                        nc.sync.dma_start(kt_sb[:], kt_dram[h])
                        nc.sync.dma_start(
                            v_sb[:],
                            v_dram[:, :, h * DK:(h + 1) * DK].rearrange(
                                "sb s d -> s sb d"),
                        )
                        # prefetch o_proj weights chunk-by-chunk behind the
                        # per-head loads so phase 3 starts with wo resident
                        nc.sync.dma_start(wo_sb[:, h], woT_t[:, h])

                        for qb in range(NQB):
                            kc_n = (qb + 1) * NDIAG
                            ps_d = den_ps.tile([1, QB], F32, tag="den")
                            ps_o = pv_ps.tile([P, QB], F32, tag="pv")

                            # software-pipelined by one kc: the scores matmul
                            # for kc+1 is issued before denom/PV of kc so the
                            # exp (ACT) latency hides behind PE work (PE engine
                            # queue is in-order).  For diagonal chunks (j >= 1)
                            # only the q >= 128j suffix is causally valid, so
                            # scores/exp/denom/PV operate on the suffix only;
                            # the 128-col triangle gets the single mask tile.
                            def off_of(kc):
                                j = kc - qb * NDIAG
                                return P * j if j > 0 else 0

                            def scores_exp(kc):
                                off = off_of(kc)
                                ps_s = sc_ps.tile([P, QB], F32, tag="sc",
                                                  name=f"ss_{h}_{qb}_{kc}")
                                nc.tensor.matmul(
                                    ps_s[:, off:],
                                    kt_sb[:, kc * P:(kc + 1) * P],
                                    qt_sb[:, qb * QB + off:(qb + 1) * QB],
                                    start=True, stop=True,
                                )
                                e_kc = expt.tile([P, QB], F32R, tag="e",
                                                 name=f"e_{h}_{qb}_{kc}")
                                nc.scalar.activation(e_kc[:, off:],
                                                     ps_s[:, off:], EXPF,
                                                     scale=SCALE)
                                j = kc - qb * NDIAG
                                if j >= 0:
                                    nc.vector.tensor_tensor(
                                        e_kc[:, off:off + P],
                                        e_kc[:, off:off + P],
                                        mask_sb[:], MUL,
                                    )
                                return e_kc

                            def denom_pv(kc, e_kc):
                                off = off_of(kc)
                                nc.tensor.matmul(
                                    ps_d[:, off:], ones_sb[:], e_kc[:, off:],
                                    start=(kc == 0), stop=(kc == kc_n - 1),
                                )
                                nc.tensor.matmul(
                                    ps_o[:, off:], v_sb[:, kc, :],
                                    e_kc[:, off:],
                                    start=(kc == 0), stop=(kc == kc_n - 1),
                                )

                            e_prev = scores_exp(0)
                            for kc in range(1, kc_n):
                                e_cur = scores_exp(kc)
                                denom_pv(kc - 1, e_prev)
                                e_prev = e_cur
                            denom_pv(kc_n - 1, e_prev)
                            inv_d = invp.tile([1, QB], F32, tag="inv")
                            nc.vector.reciprocal(inv_d[:], ps_d[:])
                            inv_b = invp.tile([P, QB], F32, tag="invb")
                            nc.gpsimd.partition_broadcast(inv_b[:], inv_d[:])
                            nc.vector.tensor_tensor(
                                attnT[:, h, qb * QB:(qb + 1) * QB],
                                ps_o[:],
                                inv_b[:],
                                MUL,
                            )

                # ---------------- Phase 3: o_proj (partial) -----------------
                with (
                    tc.tile_pool(name="op_ps", bufs=4, space="PSUM") as op_ps,
                    tc.tile_pool(name="osb", bufs=3) as osb,
                ):
                    for sb_i in range(NSB):
                        for nt in range(D // 512):
                            ps = op_ps.tile([P, 512], F32, tag="op",
                                            name=f"op_{sb_i}_{nt}")
                            for ec in range(H_loc):
                                nc.tensor.matmul(
                                    ps[:],
                                    attnT[:, ec, sb_i * P:(sb_i + 1) * P],
                                    wo_sb[:, ec, nt * 512:(nt + 1) * 512],
                                    start=(ec == 0), stop=(ec == H_loc - 1),
                                )
                            o_nt = osb.tile([P, 512], F32, tag="osb",
                                            name=f"osb_{sb_i}_{nt}")
                            nc.scalar.activation(o_nt[:], ps[:], COPYF)
                            nc.gpsimd.dma_start(
                                out[sb_i * P:(sb_i + 1) * P,
                                    nt * 512:(nt + 1) * 512],
                                o_nt[:],
                            )


    nc.compile()
    return nc


def make_tables(token_positions, S_=S, DK=D_K):
    """Host-side RoPE tables (de-interleaved halves) + causal mask tiles."""
    pos = np.asarray(token_positions).astype(np.float64)
    half = np.arange(0, DK, 2, dtype=np.float64) / DK
    inv_freq = 1.0 / (ROPE_THETA ** half)  # [DK/2]
    ang = pos[:, None] * inv_freq[None, :]  # [S, DK/2]
    c = np.cos(ang).T.astype(np.float32)  # [DK/2, S]
    s = np.sin(ang).T.astype(np.float32)
    cosH = np.ascontiguousarray(np.concatenate([c, c], axis=0))  # [DK, S]
    sinH = np.ascontiguousarray(np.concatenate([s, s], axis=0))
    kl = np.arange(128)[:, None]
    ql = np.arange(128)[None, :]
    masks = (ql >= kl).astype(np.float32)  # [128, 128] causal triangle
    return cosH, sinH, masks


# de-interleave permutation within each head's 128 dims: even dims first
_DEINT = np.concatenate([np.arange(0, D_K, 2), np.arange(1, D_K, 2)])


def deinterleave_cols(wT, n_heads):
    """Permute per-head output columns of a [D, n_heads*DK] matrix so even
    RoPE dims land in rows 0..63 of the head-transposed projection."""
    w = np.asarray(wT)
    out = np.empty_like(w)
    for h in range(n_heads):
        out[:, h * D_K:(h + 1) * D_K] = w[:, h * D_K + _DEINT]
    return out


def make_in_maps(x, token_positions, q_w, k_w, v_w, o_w):
    cosH, sinH, masks = make_tables(token_positions)
    x = np.asarray(x, np.float32)
    in_maps = []
    for c in range(N_CORES):
        b, g = c // GROUPS, c % GROUPS
        e_lo, e_hi = g * H_LOC * D_K, (g + 1) * H_LOC * D_K
        wqT = np.asarray(q_w, np.float32)[e_lo:e_hi, :].T
        wkT = np.asarray(k_w, np.float32)[e_lo:e_hi, :].T
        in_maps.append({
            "xT": np.ascontiguousarray(x[b].T),
            "wqT": np.ascontiguousarray(deinterleave_cols(wqT, H_LOC)),
            "wkT": np.ascontiguousarray(deinterleave_cols(wkT, H_LOC)),
            "wvT": np.ascontiguousarray(np.asarray(v_w, np.float32)[e_lo:e_hi, :].T),
            "woT": np.ascontiguousarray(np.asarray(o_w, np.float32)[:, e_lo:e_hi].T),
            "cosH": cosH,
            "sinH": sinH,
            "masks": masks,
            "ones": np.ones((128, 1), np.float32),
        })
    return in_maps


_NC_CACHE = None


def get_nc():
    global _NC_CACHE
    if _NC_CACHE is None:
        _NC_CACHE = build_nc(D_MODEL, S, H_LOC)
    return _NC_CACHE


def kernel(x, token_positions, q_w, k_w, v_w, o_w):
    from concourse.bass_utils import run_bass_kernel_spmd

    nc = get_nc()
    in_maps = make_in_maps(x, token_positions, q_w, k_w, v_w, o_w)
    res = run_bass_kernel_spmd(nc, in_maps, list(range(N_CORES)))
    outs = [res.results[c]["out"] for c in range(N_CORES)]
    full = np.empty((B, S, D_MODEL), np.float32)
    for b in range(B):
        full[b] = outs[GROUPS * b]
        for g in range(1, GROUPS):
            full[b] += outs[GROUPS * b + g]
    return full

